# revision 8
# baseline (speedup 1.0000x reference)
"""GAU (Gated Attention Unit, relu^2 attention) Trainium2 Bass kernel.

Problem shapes: x [4, 2048, 2048] f32; W_hidden [2048, 8192]; W_qk [2048, 128];
W_out [4096, 2048]; out = GAU(x) + x.

Sharding (8 cores): core = 2*batch + h, h in {0,1}.  Each pair of cores
handles one batch; within the pair the hidden dim (v cols 4096, gate cols
4096) is column-split in half (h picks cols [h*2048:(h+1)*2048] of v and of
gate, and rows [h*2048:(h+1)*2048] of W_out).  The qk projection (128 wide)
and the 2048x2048 attention matrix are replicated within the pair (~3% extra
flops).  Each core produces a partial output [2048, 2048] (its W_out-half
contribution) with the residual x pre-added on the rows it owns; a pairwise
ReduceScatter(add) then leaves each core with its final [1024, 2048] row
block, which the host concatenates.

Dataflow per core (all matmuls bf16 operands, fp32 PSUM accumulation; the
branch contribution is ~5e-4 of the output scale, so bf16 keeps overall
relative error ~1e-5):

  xT   [d, i]  (host-pretransposed bf16)
  qkT  [e, i] = silu(Wqk^T x^T + b_qk)        lhsT=Wqk[d,e],  rhs=xT[d,i]
  qT/kT = gamma*qkT + beta (per-partition affine)
  v    [j, c] = silu(x Whv + b_hv)            lhsT=xT[d,j],   rhs=Whv[d,c]
  gateT[c, i] = silu(Whg^T x^T + b_hg)        lhsT=Whg[d,c],  rhs=xT[d,i]   (spilled to DRAM)
  attnT[j, i] = relu(qkT^T qkT / seq)^2       lhsT=kT[e,j],   rhs=qT[e,i]
  ogT  [c, i] = (v^T attnT) * gateT           lhsT=v[j,c],    rhs=attnT[j,i]
  part [i, d] = ogT^T Wout_h (+b_out +x_own)  lhsT=ogT[c,i],  rhs=Wout[c,d]

No on-device transposes are needed: every contraction has both operands
naturally laid out with the contraction dim on partitions.  Free-dim biases
(b_hidden v-part, b_out) are added with K=1 rank-1 matmuls into PSUM;
partition-dim biases (b_qk, b_hidden gate-part) use the activation bias port.
"""

import numpy as np
import ml_dtypes
from contextlib import ExitStack

import concourse.bass as bass
import concourse.bacc as bacc
import concourse.mybir as mybir
import concourse.tile as tile
from concourse.bass_utils import run_bass_kernel_spmd

BF16 = mybir.dt.bfloat16
F32 = mybir.dt.float32
AF = mybir.ActivationFunctionType
ALU = mybir.AluOpType
P = 128


def build_gau_nc(seq=2048, dim=2048, hh=2048, n_cores=8,
                 IC=None, CC=None, DC=None):
    """Build the SPMD program.  hh = per-core hidden half width."""
    e = P  # qk dim
    nd = dim // P       # d-tiles (contraction tiles for x)
    njt = seq // P      # seq tiles (j)
    IC = IC or min(512, seq)  # i-chunk (moving free dim)
    n_ic = seq // IC
    CC = CC or min(512, hh)   # c-chunk for v
    n_cc = hh // CC
    nct = hh // P       # c-tiles
    DC = DC or min(512, dim)  # d-chunk for the output matmul
    n_dc = dim // DC
    n_it = IC // P      # i-tiles per i-chunk
    pairs = [[2 * g, 2 * g + 1] for g in range(n_cores // 2)]

    nc = bacc.Bacc("TRN2", target_bir_lowering=False, debug=False,
                   num_devices=n_cores)

    xT_d = nc.dram_tensor("xT", [dim, seq], BF16, kind="ExternalInput")
    whv_d = nc.dram_tensor("whv", [dim, hh], BF16, kind="ExternalInput")
    whg_d = nc.dram_tensor("whg", [dim, hh], BF16, kind="ExternalInput")
    wqk_d = nc.dram_tensor("wqk", [dim, e], BF16, kind="ExternalInput")
    wout_d = nc.dram_tensor("wout", [hh, dim], BF16, kind="ExternalInput")
    bqk_d = nc.dram_tensor("bqk", [e, 1], F32, kind="ExternalInput")
    gq_d = nc.dram_tensor("gq", [e, 1], F32, kind="ExternalInput")
    bq_d = nc.dram_tensor("bq", [e, 1], F32, kind="ExternalInput")
    gk_d = nc.dram_tensor("gk", [e, 1], F32, kind="ExternalInput")
    bk_d = nc.dram_tensor("bk", [e, 1], F32, kind="ExternalInput")
    bhv_d = nc.dram_tensor("bhv", [1, hh], BF16, kind="ExternalInput")
    bhgT_d = nc.dram_tensor("bhgT", [P, nct], F32, kind="ExternalInput")
    bout_d = nc.dram_tensor("bout", [1, dim], BF16, kind="ExternalInput")
    xres_d = nc.dram_tensor("xres", [seq, dim], F32, kind="ExternalInput")
    out_d = nc.dram_tensor("out", [seq // 2, dim], F32, kind="ExternalOutput")

    with TileCtx(nc) as tc, ExitStack() as st:
        constp = st.enter_context(tc.tile_pool(name="const", bufs=1))
        psump = st.enter_context(tc.tile_pool(name="psum", bufs=8, space="PSUM"))
        dramp = st.enter_context(tc.tile_pool(name="dram", bufs=1, space="DRAM"))
        mainp = st.enter_context(tc.tile_pool(name="main", bufs=1))

        gtd = dramp.tile([hh, seq], F32, tag="gtd", name="gtd")           # gateT spill
        partial = dramp.tile([seq, dim], F32, tag="partial", name="partial")  # pre-reduce out
        red = dramp.tile([seq // 2, dim], F32, tag="red", name="red")     # post-RS

        # ---- constants ----
        wqk_sb = constp.tile([P, nd * e], BF16, tag="wqk")
        for d in range(nd):
            nc.sync.dma_start(wqk_sb[:, d * e:(d + 1) * e],
                              wqk_d[d * P:(d + 1) * P, :])
        bqk_sb = constp.tile([e, 1], F32, tag="bqk")
        nc.sync.dma_start(bqk_sb[:], bqk_d[:])
        gq_sb = constp.tile([e, 1], F32, tag="gq")
        nc.sync.dma_start(gq_sb[:], gq_d[:])
        bq_sb = constp.tile([e, 1], F32, tag="bq")
        nc.sync.dma_start(bq_sb[:], bq_d[:])
        gk_sb = constp.tile([e, 1], F32, tag="gk")
        nc.sync.dma_start(gk_sb[:], gk_d[:])
        bk_sb = constp.tile([e, 1], F32, tag="bk")
        nc.sync.dma_start(bk_sb[:], bk_d[:])
        bhgT_sb = constp.tile([P, nct], F32, tag="bhgT")
        nc.sync.dma_start(bhgT_sb[:], bhgT_d[:])
        bhv_sb = constp.tile([1, hh], BF16, tag="bhv")
        nc.sync.dma_start(bhv_sb[:], bhv_d[:])
        bout_sb = constp.tile([1, dim], BF16, tag="bout")
        nc.sync.dma_start(bout_sb[:], bout_d[:])
        ones_sb = constp.tile([1, P], BF16, tag="ones")
        nc.vector.memset(ones_sb[:], 1.0)

        # persistent activations
        qT_sb = mainp.tile([e, seq], BF16, tag="qT", name="qT")
        kT_sb = mainp.tile([e, seq], BF16, tag="kT", name="kT")
        v_sb = [mainp.tile([P, hh], BF16, tag=f"v{jt}", name=f"v{jt}") for jt in range(njt)]

        with tc.tile_pool(name="ph1", bufs=1) as ph1p, \
             tc.tile_pool(name="wstream", bufs=1) as wsp:
            xT_sb = [ph1p.tile([P, seq], BF16, tag=f"xT{d}", name=f"xT{d}") for d in range(nd)]
            for d in range(nd):
                nc.sync.dma_start(xT_sb[d][:], xT_d[d * P:(d + 1) * P, :])

            # ---- qk projection ----
            # silu(u) = u * sigmoid(u); the interp has no Silu LUT, so build
            # it from Sigmoid (ACT) + mult (DVE) everywhere.
            with tc.tile_pool(name="qkp", bufs=1) as qkp:
                qk_sb = qkp.tile([e, seq], F32, tag="qk", name="qk")
                for ic in range(n_ic):
                    ps = psump.tile([P, IC], F32, tag="ps", name="ps")
                    for d in range(nd):
                        nc.tensor.matmul(ps[:], wqk_sb[:, d * e:(d + 1) * e],
                                         xT_sb[d][:, ic * IC:(ic + 1) * IC],
                                         start=(d == 0), stop=(d == nd - 1))
                    sg = qkp.tile([P, IC], F32, tag="sg1", bufs=2, name="sg")
                    nc.scalar.activation(sg[:], ps[:], AF.Sigmoid,
                                         bias=bqk_sb[:])
                    u = qkp.tile([P, IC], F32, tag="u1", bufs=2, name="u")
                    nc.vector.tensor_scalar_add(u[:], ps[:], bqk_sb[:])
                    nc.vector.tensor_tensor(qk_sb[:, ic * IC:(ic + 1) * IC],
                                            u[:], sg[:], ALU.mult)
                nc.vector.tensor_scalar(qT_sb[:], qk_sb[:], gq_sb[:],
                                        bq_sb[:], ALU.mult, ALU.add)
                nc.vector.tensor_scalar(kT_sb[:], qk_sb[:], gk_sb[:],
                                        bk_sb[:], ALU.mult, ALU.add)

            # ---- hidden, v part: v[j, c] ----
            for cc in range(n_cc):
                wv = [wsp.tile([P, CC], BF16, tag=f"wv{d}", bufs=1, name=f"wv{d}")
                      for d in range(nd)]
                for d in range(nd):
                    nc.sync.dma_start(wv[d][:],
                                      whv_d[d * P:(d + 1) * P,
                                            cc * CC:(cc + 1) * CC])
                for jt in range(njt):
                    ps = psump.tile([P, CC], F32, tag="ps", name="ps")
                    for d in range(nd):
                        nc.tensor.matmul(ps[:], xT_sb[d][:, jt * P:(jt + 1) * P],
                                         wv[d][:], start=(d == 0), stop=False)
                    nc.tensor.matmul(ps[:], ones_sb[:],
                                     bhv_sb[:, cc * CC:(cc + 1) * CC],
                                     start=False, stop=True)
                    sg = wsp.tile([P, CC], F32, tag="sgv", bufs=2, name="sgv")
                    nc.scalar.activation(sg[:], ps[:], AF.Sigmoid)
                    nc.vector.tensor_tensor(v_sb[jt][:, cc * CC:(cc + 1) * CC],
                                            ps[:], sg[:], ALU.mult)

            # ---- hidden, gate part: gateT[c, i] -> DRAM ----
            for ct in range(nct):
                wg = [wsp.tile([P, P], BF16, tag=f"wg{d}", bufs=2, name=f"wg{d}")
                      for d in range(nd)]
                for d in range(nd):
                    nc.sync.dma_start(wg[d][:],
                                      whg_d[d * P:(d + 1) * P,
                                            ct * P:(ct + 1) * P])
                for ic in range(n_ic):
                    ps = psump.tile([P, IC], F32, tag="ps", name="ps")
                    for d in range(nd):
                        nc.tensor.matmul(ps[:], wg[d][:],
                                         xT_sb[d][:, ic * IC:(ic + 1) * IC],
                                         start=(d == 0), stop=(d == nd - 1))
                    sg = wsp.tile([P, IC], F32, tag="sgg", bufs=2, name="sgg")
                    nc.scalar.activation(sg[:], ps[:], AF.Sigmoid,
                                         bias=bhgT_sb[:, ct:ct + 1])
                    u = wsp.tile([P, IC], F32, tag="ug", bufs=2, name="ug")
                    nc.vector.tensor_scalar_add(u[:], ps[:],
                                                bhgT_sb[:, ct:ct + 1])
                    gstage = wsp.tile([P, IC], F32, tag="gstage", bufs=2, name="gstage")
                    nc.vector.tensor_tensor(gstage[:], u[:], sg[:], ALU.mult)
                    nc.sync.dma_start(gtd[ct * P:(ct + 1) * P,
                                          ic * IC:(ic + 1) * IC], gstage[:])

        # ---- attention + output, per i-chunk ----
        with tc.tile_pool(name="ph2", bufs=1) as ph2p:
            at_sb = [ph2p.tile([P, IC], BF16, tag=f"at{jt}", name=f"at{jt}") for jt in range(njt)]
            og_sb = [ph2p.tile([P, IC], BF16, tag=f"og{ct}", name=f"og{ct}") for ct in range(nct)]
            for ic in range(n_ic):
                # attnT[j, ic] = relu(sim/seq)^2, bf16
                for jt in range(njt):
                    ps = psump.tile([P, IC], F32, tag="ps", name="ps")
                    nc.tensor.matmul(ps[:], kT_sb[:, jt * P:(jt + 1) * P],
                                     qT_sb[:, ic * IC:(ic + 1) * IC],
                                     start=True, stop=True)
                    rstage = ph2p.tile([P, IC], F32, tag="rstage", bufs=4, name="rstage")
                    nc.scalar.activation(rstage[:], ps[:], AF.Relu,
                                         scale=1.0 / seq)
                    nc.vector.tensor_tensor(at_sb[jt][:], rstage[:], rstage[:],
                                            ALU.mult)
                # ogT[c, ic] = (v^T @ attnT) * gateT
                for ct in range(nct):
                    ps = psump.tile([P, IC], F32, tag="ps", name="ps")
                    for jt in range(njt):
                        nc.tensor.matmul(ps[:], v_sb[jt][:, ct * P:(ct + 1) * P],
                                         at_sb[jt][:],
                                         start=(jt == 0), stop=(jt == njt - 1))
                    g = ph2p.tile([P, IC], F32, tag="g", bufs=4, name="g")
                    nc.sync.dma_start(g[:], gtd[ct * P:(ct + 1) * P,
                                                ic * IC:(ic + 1) * IC])
                    nc.vector.tensor_tensor(og_sb[ct][:], ps[:], g[:], ALU.mult)
                # partial[ic rows, :] = ogT^T @ Wout (+ b_out + x_masked)
                for dc in range(n_dc):
                    wo = [ph2p.tile([P, DC], BF16, tag=f"wo{ct}", bufs=2, name=f"wo{ct}")
                          for ct in range(nct)]
                    for ct in range(nct):
                        nc.sync.dma_start(wo[ct][:],
                                          wout_d[ct * P:(ct + 1) * P,
                                                 dc * DC:(dc + 1) * DC])
                    for it in range(n_it):
                        i_abs = ic * n_it + it
                        ps = psump.tile([P, DC], F32, tag="ps", name="ps")
                        for ct in range(nct):
                            nc.tensor.matmul(ps[:],
                                             og_sb[ct][:, it * P:(it + 1) * P],
                                             wo[ct][:],
                                             start=(ct == 0), stop=False)
                        nc.tensor.matmul(ps[:], ones_sb[:],
                                         bout_sb[:, dc * DC:(dc + 1) * DC],
                                         start=False, stop=True)
                        xr = ph2p.tile([P, DC], F32, tag="xr", bufs=4, name="xr")
                        nc.sync.dma_start(xr[:],
                                          xres_d[i_abs * P:(i_abs + 1) * P,
                                                 dc * DC:(dc + 1) * DC])
                        po = ph2p.tile([P, DC], F32, tag="po", bufs=4, name="po")
                        nc.vector.tensor_tensor(po[:], ps[:], xr[:], ALU.add)
                        nc.sync.dma_start(partial[i_abs * P:(i_abs + 1) * P,
                                                  dc * DC:(dc + 1) * DC], po[:])

        # ---- pairwise reduce-scatter + output ----
        nc.gpsimd.collective_compute("ReduceScatter", ALU.add,
                                     replica_groups=pairs,
                                     ins=[partial.opt()], outs=[red.opt()])
        half = seq // 2
        for t in range(half // P):
            nc.sync.dma_start(out_d[t * P:(t + 1) * P, :],
                              red[t * P:(t + 1) * P, :])

    nc.compile()
    return nc


def TileCtx(nc):
    return tile.TileContext(nc)


def make_in_maps(x, W_hidden, b_hidden, W_qk, b_qk, gamma_q, beta_q,
                 gamma_k, beta_k, W_out, b_out, n_cores=8):
    """Host-side sharding/layout prep.  Returns per-core input dicts."""
    bf = ml_dtypes.bfloat16
    B, seq, dim = x.shape
    H2 = W_hidden.shape[1]
    H = H2 // 2
    hh = H // 2  # per-core half of v (and of gate)
    nct = hh // P
    in_maps = []
    xT_cache = {}
    for core in range(n_cores):
        b, h = core // 2, core % 2
        if b not in xT_cache:
            xT_cache[b] = np.ascontiguousarray(x[b].T).astype(bf)
        xres = np.zeros((seq, dim), np.float32)
        if h == 0:
            xres[: seq // 2] = x[b][: seq // 2]
        else:
            xres[seq // 2:] = x[b][seq // 2:]
        cs = slice(h * hh, (h + 1) * hh)
        gs = slice(H + h * hh, H + (h + 1) * hh)
        in_maps.append({
            "xT": xT_cache[b],
            "whv": W_hidden[:, cs].astype(bf),
            "whg": W_hidden[:, gs].astype(bf),
            "wqk": W_qk.astype(bf),
            "wout": W_out[cs, :].astype(bf),
            "bqk": b_qk.reshape(-1, 1).astype(np.float32),
            "gq": gamma_q.reshape(-1, 1).astype(np.float32),
            "bq": beta_q.reshape(-1, 1).astype(np.float32),
            "gk": gamma_k.reshape(-1, 1).astype(np.float32),
            "bk": beta_k.reshape(-1, 1).astype(np.float32),
            "bhv": b_hidden[cs].reshape(1, -1).astype(bf),
            "bhgT": np.ascontiguousarray(
                b_hidden[gs].reshape(nct, P).T).astype(np.float32),
            "bout": (b_out if h == 0 else np.zeros_like(b_out)
                     ).reshape(1, -1).astype(bf),
            "xres": xres,
        })
    return in_maps


_NC_CACHE = {}


def _get_nc(seq, dim, hh, n_cores):
    key = (seq, dim, hh, n_cores)
    if key not in _NC_CACHE:
        _NC_CACHE[key] = build_gau_nc(seq=seq, dim=dim, hh=hh, n_cores=n_cores)
    return _NC_CACHE[key]


def kernel(x, W_hidden, b_hidden, W_qk, b_qk, gamma_q, beta_q, gamma_k,
           beta_k, W_out, b_out):
    x = np.asarray(x)
    B, seq, dim = x.shape
    hh = W_hidden.shape[1] // 4
    n_cores = 2 * B
    nc = _get_nc(seq, dim, hh, n_cores)
    in_maps = make_in_maps(x, np.asarray(W_hidden), np.asarray(b_hidden),
                           np.asarray(W_qk), np.asarray(b_qk),
                           np.asarray(gamma_q), np.asarray(beta_q),
                           np.asarray(gamma_k), np.asarray(beta_k),
                           np.asarray(W_out), np.asarray(b_out),
                           n_cores=n_cores)
    res = run_bass_kernel_spmd(nc, in_maps, core_ids=list(range(n_cores)))
    out = np.empty((B, seq, dim), np.float32)
    half = seq // 2
    for b in range(B):
        out[b, :half] = res.results[2 * b]["out"]
        out[b, half:] = res.results[2 * b + 1]["out"]
    return out


# revision 21
# speedup vs baseline: 1.2193x; 1.2193x over previous
"""GAU (Gated Attention Unit, relu^2 attention) Trainium2 Bass kernel.

Problem shapes: x [4, 2048, 2048] f32; W_hidden [2048, 8192]; W_qk [2048, 128];
W_out [4096, 2048]; out = GAU(x) + x.

Sharding (8 cores): core = 2*batch + h, h in {0,1}.  Each pair of cores
handles one batch; within the pair the hidden dim (v cols 4096, gate cols
4096) is column-split in half (h picks cols [h*2048:(h+1)*2048] of v and of
gate, and rows [h*2048:(h+1)*2048] of W_out).  The qk projection (128 wide)
and the 2048x2048 attention matrix are replicated within the pair (~3% extra
flops).  Each core produces a partial output [2048, 2048] (its W_out-half
contribution) with the residual x pre-added on the rows it owns; a pairwise
ReduceScatter(add) then leaves each core with its final [1024, 2048] row
block, which the host concatenates.

Dataflow per core (all matmuls bf16 operands, fp32 PSUM accumulation; the
branch contribution is ~5e-4 of the output scale, so bf16 keeps overall
relative error ~1e-5):

  xT   [d, i]  (host-pretransposed bf16)
  qkT  [e, i] = silu(Wqk^T x^T + b_qk)        lhsT=Wqk[d,e],  rhs=xT[d,i]
  qT/kT = gamma*qkT + beta (per-partition affine)
  v    [j, c] = silu(x Whv + b_hv)            lhsT=xT[d,j],   rhs=Whv[d,c]
  gateT[c, i] = silu(Whg^T x^T + b_hg)        lhsT=Whg[d,c],  rhs=xT[d,i]   (spilled to DRAM)
  attnT[j, i] = relu(qkT^T qkT / seq)^2       lhsT=kT[e,j],   rhs=qT[e,i]
  ogT  [c, i] = (v^T attnT) * gateT           lhsT=v[j,c],    rhs=attnT[j,i]
  part [i, d] = ogT^T Wout_h (+b_out +x_own)  lhsT=ogT[c,i],  rhs=Wout[c,d]

No on-device transposes are needed: every contraction has both operands
naturally laid out with the contraction dim on partitions.  Free-dim biases
(b_hidden v-part, b_out) are added with K=1 rank-1 matmuls into PSUM;
partition-dim biases (b_qk, b_hidden gate-part) use the activation bias port.
"""

import numpy as np
import ml_dtypes
from contextlib import ExitStack

import concourse.bass as bass
import concourse.bacc as bacc
import concourse.mybir as mybir
import concourse.tile as tile
from concourse.bass_utils import run_bass_kernel_spmd

BF16 = mybir.dt.bfloat16
F32 = mybir.dt.float32
AF = mybir.ActivationFunctionType
ALU = mybir.AluOpType
P = 128


def build_gau_nc(seq=2048, dim=2048, hh=2048, n_cores=8,
                 IC=None, CC=None, DC=None, with_bhv=True):
    """Build the SPMD program.  hh = per-core hidden half width."""
    e = P  # qk dim
    nd = dim // P       # d-tiles (contraction tiles for x)
    njt = seq // P      # seq tiles (j)
    IC = IC or min(512, seq)  # i-chunk (moving free dim)
    n_ic = seq // IC
    CC = CC or min(512, hh)   # c-chunk for v
    n_cc = hh // CC
    nct = hh // P       # c-tiles
    DC = DC or min(512, dim)  # d-chunk for the output matmul
    n_dc = dim // DC
    n_it = IC // P      # i-tiles per i-chunk
    pairs = [[2 * g, 2 * g + 1] for g in range(n_cores // 2)]

    nc = bacc.Bacc("TRN2", target_bir_lowering=False, debug=False,
                   num_devices=n_cores)

    xT_d = nc.dram_tensor("xT", [dim, seq], BF16, kind="ExternalInput")
    whv_d = nc.dram_tensor("whv", [dim, hh], BF16, kind="ExternalInput")
    whg_d = nc.dram_tensor("whg", [dim, hh], BF16, kind="ExternalInput")
    wqk_d = nc.dram_tensor("wqk", [P, dim], BF16, kind="ExternalInput")
    wout_d = nc.dram_tensor("wout", [hh, dim], BF16, kind="ExternalInput")
    bqk_d = nc.dram_tensor("bqk", [e, 1], F32, kind="ExternalInput")
    gq_d = nc.dram_tensor("gq", [e, 1], F32, kind="ExternalInput")
    bq_d = nc.dram_tensor("bq", [e, 1], F32, kind="ExternalInput")
    gk_d = nc.dram_tensor("gk", [e, 1], F32, kind="ExternalInput")
    bk_d = nc.dram_tensor("bk", [e, 1], F32, kind="ExternalInput")
    bhv_d = nc.dram_tensor("bhv", [1, hh], BF16, kind="ExternalInput")
    bhgT_d = nc.dram_tensor("bhgT", [P, nct], F32, kind="ExternalInput")
    xres_d = nc.dram_tensor("xres", [seq // 2, dim], F32, kind="ExternalInput")
    out_d = nc.dram_tensor("out", [seq // 2, dim], F32, kind="ExternalOutput")

    with TileCtx(nc) as tc, ExitStack() as st:
        constp = st.enter_context(tc.tile_pool(name="const", bufs=1))
        psump = st.enter_context(tc.tile_pool(name="psum", bufs=8, space="PSUM"))
        dramp = st.enter_context(tc.tile_pool(name="dram", bufs=1, space="DRAM"))
        mainp = st.enter_context(tc.tile_pool(name="main", bufs=1))

        gtd = dramp.tile([hh, seq], BF16, tag="gtd", name="gtd")  # gateT spill
        # per-128-row-block reduce buffers: each block's ReduceScatter can
        # launch as soon as its partial rows are written (overlaps compute)
        # and the final (serial) reduce quantum is small
        pb = [dramp.tile([P, dim], BF16, tag=f"pb{k}", name=f"pb{k}")
              for k in range(njt)]
        rb = [dramp.tile([P // 2, dim], BF16, tag=f"rb{k}", name=f"rb{k}")
              for k in range(njt)]

        # ---- constants ----
        wqk_sb = constp.tile([P, nd * e], BF16, tag="wqk")
        nc.sync.dma_start(wqk_sb[:], wqk_d[:])
        bqk_sb = constp.tile([e, 1], F32, tag="bqk")
        nc.sync.dma_start(bqk_sb[:], bqk_d[:])
        gq_sb = constp.tile([e, 1], F32, tag="gq")
        nc.sync.dma_start(gq_sb[:], gq_d[:])
        bq_sb = constp.tile([e, 1], F32, tag="bq")
        nc.sync.dma_start(bq_sb[:], bq_d[:])
        gk_sb = constp.tile([e, 1], F32, tag="gk")
        nc.sync.dma_start(gk_sb[:], gk_d[:])
        bk_sb = constp.tile([e, 1], F32, tag="bk")
        nc.sync.dma_start(bk_sb[:], bk_d[:])
        bhgT_sb = constp.tile([P, nct], F32, tag="bhgT")
        nc.sync.dma_start(bhgT_sb[:], bhgT_d[:])
        bhv_sb = constp.tile([1, hh], BF16, tag="bhv")
        nc.sync.dma_start(bhv_sb[:], bhv_d[:])
        ones_sb = constp.tile([1, P], BF16, tag="ones")
        nc.vector.memset(ones_sb[:], 1.0)

        # tiny ReduceScatter to warm the collective stream while the PE is
        # busy with the projections -- the first real RS otherwise pays a
        # ~50us cold-start that stalls the pipeline
        warm_in = dramp.tile([2, 64], F32, tag="warm_in", name="warm_in")
        warm_out = dramp.tile([1, 64], F32, tag="warm_out", name="warm_out")
        warm_sb = constp.tile([2, 64], F32, tag="warm_sb")
        nc.vector.memset(warm_sb[:], 0.0)
        nc.gpsimd.dma_start(warm_in[:], warm_sb[:])
        nc.gpsimd.collective_compute("ReduceScatter", ALU.add,
                                     replica_groups=pairs,
                                     ins=[warm_in.opt()],
                                     outs=[warm_out.opt()])

        # persistent activations
        qT_sb = mainp.tile([e, seq], BF16, tag="qT", name="qT")
        kT_sb = mainp.tile([e, seq], BF16, tag="kT", name="kT")
        v_sb = [mainp.tile([P, hh], BF16, tag=f"v{jt}", name=f"v{jt}") for jt in range(njt)]

        with tc.tile_pool(name="ph1", bufs=1) as ph1p, \
             tc.tile_pool(name="wstream", bufs=1) as wsp:
            xT_sb = [ph1p.tile([P, seq], BF16, tag=f"xT{d}", name=f"xT{d}") for d in range(nd)]
            for half in range(2):
                cols = slice(half * (seq // 2), (half + 1) * (seq // 2))
                for d in range(nd):
                    nc.sync.dma_start(xT_sb[d][:, cols],
                                      xT_d[d * P:(d + 1) * P, cols])

            # ---- qk projection ----
            # silu(u) = u * sigmoid(u); the interp has no Silu LUT, so build
            # it from Sigmoid (ACT) + mult (DVE) everywhere.
            with tc.tile_pool(name="qkp", bufs=1) as qkp:
                qk_sb = qkp.tile([e, seq], F32, tag="qk", name="qk")
                for ic in range(n_ic):
                    ps = psump.tile([P, IC], F32, tag="ps", name="ps")
                    for d in range(nd):
                        nc.tensor.matmul(ps[:], wqk_sb[:, d * e:(d + 1) * e],
                                         xT_sb[d][:, ic * IC:(ic + 1) * IC],
                                         start=(d == 0), stop=(d == nd - 1))
                    sg = qkp.tile([P, IC], F32, tag="sg1", bufs=2, name="sg")
                    nc.scalar.activation(sg[:], ps[:], AF.Sigmoid,
                                         bias=bqk_sb[:])
                    u = qkp.tile([P, IC], F32, tag="u1", bufs=2, name="u")
                    nc.vector.tensor_scalar_add(u[:], ps[:], bqk_sb[:])
                    nc.vector.tensor_tensor(qk_sb[:, ic * IC:(ic + 1) * IC],
                                            u[:], sg[:], ALU.mult)
                nc.vector.tensor_scalar(qT_sb[:], qk_sb[:], gq_sb[:],
                                        bq_sb[:], ALU.mult, ALU.add)
                nc.vector.tensor_scalar(kT_sb[:], qk_sb[:], gk_sb[:],
                                        bk_sb[:], ALU.mult, ALU.add)

            # ---- hidden, v part: v[j, c] ----
            for cc in range(n_cc):
                wv = [wsp.tile([P, CC], BF16, tag=f"wv{d}", bufs=1, name=f"wv{d}")
                      for d in range(nd)]
                for d in range(nd):
                    nc.sync.dma_start(wv[d][:],
                                      whv_d[d * P:(d + 1) * P,
                                            cc * CC:(cc + 1) * CC])
                for jt in range(njt):
                    ps = psump.tile([P, CC], F32, tag="ps", name="ps")
                    for d in range(nd):
                        nc.tensor.matmul(ps[:], xT_sb[d][:, jt * P:(jt + 1) * P],
                                         wv[d][:], start=(d == 0),
                                         stop=(not with_bhv and d == nd - 1))
                    if with_bhv:
                        nc.tensor.matmul(ps[:], ones_sb[:],
                                         bhv_sb[:, cc * CC:(cc + 1) * CC],
                                         start=False, stop=True)
                    sg = wsp.tile([P, CC], F32, tag="sgv", bufs=2, name="sgv")
                    nc.scalar.activation(sg[:], ps[:], AF.Sigmoid)
                    nc.vector.tensor_tensor(v_sb[jt][:, cc * CC:(cc + 1) * CC],
                                            ps[:], sg[:], ALU.mult)

            # ---- hidden, gate part: gateT[c, i] -> DRAM ----
            for ct in range(nct):
                wg = [wsp.tile([P, P], BF16, tag=f"wg{d}", bufs=2, name=f"wg{d}")
                      for d in range(nd)]
                for d in range(nd):
                    nc.sync.dma_start(wg[d][:],
                                      whg_d[d * P:(d + 1) * P,
                                            ct * P:(ct + 1) * P])
                for ic in range(n_ic):
                    ps = psump.tile([P, IC], F32, tag="ps", name="ps")
                    for d in range(nd):
                        nc.tensor.matmul(ps[:], wg[d][:],
                                         xT_sb[d][:, ic * IC:(ic + 1) * IC],
                                         start=(d == 0), stop=(d == nd - 1))
                    sg = wsp.tile([P, IC], F32, tag="sgg", bufs=2, name="sgg")
                    nc.scalar.activation(sg[:], ps[:], AF.Sigmoid,
                                         bias=bhgT_sb[:, ct:ct + 1])
                    u = wsp.tile([P, IC], F32, tag="ug", bufs=2, name="ug")
                    nc.vector.tensor_scalar_add(u[:], ps[:],
                                                bhgT_sb[:, ct:ct + 1])
                    gstage = wsp.tile([P, IC], BF16, tag="gstage", bufs=2, name="gstage")
                    nc.vector.tensor_tensor(gstage[:], u[:], sg[:], ALU.mult)
                    nc.scalar.dma_start(gtd[ct * P:(ct + 1) * P,
                                            ic * IC:(ic + 1) * IC], gstage[:])

        # ---- attention + output, per i-chunk ----
        # chunk widths: full IC chunks, with the last split (IC-P, P) so the
        # final (serial) block reduce covers only P rows and earlier blocks'
        # reduces hide under the small chunk's compute
        widths = [IC] * n_ic
        with tc.tile_pool(name="ph2", bufs=1) as ph2p:
            at_sb = [ph2p.tile([P, IC], BF16, tag=f"at{jt}", name=f"at{jt}") for jt in range(njt)]
            og_sb = [ph2p.tile([P, IC], BF16, tag=f"og{ct}", name=f"og{ct}") for ct in range(nct)]
            cstart = 0
            for cw in widths:
                n_it_c = cw // P
                # attnT[j, chunk] = relu(sim/seq)^2, bf16
                for jt in range(njt):
                    ps = psump.tile([P, cw], F32, tag="ps", name="ps")
                    nc.tensor.matmul(ps[:], kT_sb[:, jt * P:(jt + 1) * P],
                                     qT_sb[:, cstart:cstart + cw],
                                     start=True, stop=True)
                    rstage = ph2p.tile([P, cw], F32, tag="rstage", bufs=4, name="rstage")
                    nc.scalar.activation(rstage[:], ps[:], AF.Relu,
                                         scale=1.0 / seq)
                    nc.vector.tensor_tensor(at_sb[jt][:, :cw], rstage[:],
                                            rstage[:], ALU.mult)
                # ogT[c, chunk] = (v^T @ attnT) * gateT
                for ct in range(nct):
                    ps = psump.tile([P, cw], F32, tag="ps", name="ps")
                    for jt in range(njt):
                        nc.tensor.matmul(ps[:], v_sb[jt][:, ct * P:(ct + 1) * P],
                                         at_sb[jt][:, :cw],
                                         start=(jt == 0), stop=(jt == njt - 1))
                    g = ph2p.tile([P, cw], BF16, tag="g", bufs=4, name="g")
                    nc.sync.dma_start(g[:], gtd[ct * P:(ct + 1) * P,
                                                cstart:cstart + cw])
                    nc.vector.tensor_tensor(og_sb[ct][:, :cw], ps[:], g[:],
                                            ALU.mult)
                # partial[chunk rows, :] = ogT^T @ Wout
                for dc in range(n_dc):
                    wo = [ph2p.tile([P, DC], BF16, tag=f"wo{ct}", bufs=2, name=f"wo{ct}")
                          for ct in range(nct)]
                    for ct in range(nct):
                        nc.sync.dma_start(wo[ct][:],
                                          wout_d[ct * P:(ct + 1) * P,
                                                 dc * DC:(dc + 1) * DC])
                    for it in range(n_it_c):
                        i_abs = cstart // P + it
                        ps = psump.tile([P, DC], F32, tag="ps", name="ps")
                        for ct in range(nct):
                            nc.tensor.matmul(ps[:],
                                             og_sb[ct][:, it * P:(it + 1) * P],
                                             wo[ct][:],
                                             start=(ct == 0),
                                             stop=(ct == nct - 1))
                        po = ph2p.tile([P, DC], BF16, tag="po", bufs=4, name="po")
                        nc.vector.tensor_copy(po[:], ps[:])
                        nc.scalar.dma_start(
                            pb[i_abs][:, dc * DC:(dc + 1) * DC], po[:])
                # pairwise reduce-scatter per 128-row block; core h of a pair
                # gets rows [h*64, (h+1)*64) of each block
                oh = P // 2
                for it in range(n_it_c):
                    k = cstart // P + it
                    nc.gpsimd.collective_compute("ReduceScatter", ALU.add,
                                                 replica_groups=pairs,
                                                 ins=[pb[k].opt()],
                                                 outs=[rb[k].opt()])
                    orow = k * oh
                    for dc in range(n_dc):
                        rd = ph2p.tile([oh, DC], BF16, tag="rd", bufs=4,
                                       name="rd")
                        nc.gpsimd.dma_start(rd[:],
                                            rb[k][:, dc * DC:(dc + 1) * DC])
                        xr = ph2p.tile([oh, DC], F32, tag="xr", bufs=4,
                                       name="xr")
                        nc.gpsimd.dma_start(xr[:],
                                            xres_d[orow:orow + oh,
                                                   dc * DC:(dc + 1) * DC])
                        fo = ph2p.tile([oh, DC], F32, tag="fo", bufs=4,
                                       name="fo")
                        nc.vector.tensor_tensor(fo[:], xr[:], rd[:], ALU.add)
                        nc.scalar.dma_start(out_d[orow:orow + oh,
                                                  dc * DC:(dc + 1) * DC],
                                            fo[:])
                cstart += cw

    nc.compile()
    return nc


def TileCtx(nc):
    return tile.TileContext(nc)


def own_rows(seq, h, IC=None):
    """Rows owned by pair-member h: half of every 128-row block (block-RS)."""
    oh = P // 2
    idx = []
    for k in range(seq // P):
        idx.extend(range(k * P + h * oh, k * P + (h + 1) * oh))
    return np.array(idx)


def make_in_maps(x, W_hidden, b_hidden, W_qk, b_qk, gamma_q, beta_q,
                 gamma_k, beta_k, W_out, b_out, n_cores=8, IC=None):
    """Host-side sharding/layout prep.  Returns per-core input dicts."""
    bf = ml_dtypes.bfloat16
    B, seq, dim = x.shape
    H2 = W_hidden.shape[1]
    H = H2 // 2
    hh = H // 2  # per-core half of v (and of gate)
    nct = hh // P
    in_maps = []
    xT_cache = {}
    for core in range(n_cores):
        b, h = core // 2, core % 2
        if b not in xT_cache:
            xT_cache[b] = np.ascontiguousarray(x[b].T).astype(bf)
        rows = own_rows(seq, h, IC)
        xres = (x[b][rows].astype(np.float32)
                + b_out.astype(np.float32)[None, :])
        cs = slice(h * hh, (h + 1) * hh)
        gs = slice(H + h * hh, H + (h + 1) * hh)
        in_maps.append({
            "xT": xT_cache[b],
            "whv": W_hidden[:, cs].astype(bf),
            "whg": W_hidden[:, gs].astype(bf),
            "wqk": np.ascontiguousarray(
                np.concatenate(np.split(W_qk.astype(bf), dim // P, axis=0),
                               axis=1)),
            "wout": W_out[cs, :].astype(bf),
            "bqk": b_qk.reshape(-1, 1).astype(np.float32),
            "gq": gamma_q.reshape(-1, 1).astype(np.float32),
            "bq": beta_q.reshape(-1, 1).astype(np.float32),
            "gk": gamma_k.reshape(-1, 1).astype(np.float32),
            "bk": beta_k.reshape(-1, 1).astype(np.float32),
            "bhv": b_hidden[cs].reshape(1, -1).astype(bf),
            "bhgT": np.ascontiguousarray(
                b_hidden[gs].reshape(nct, P).T).astype(np.float32),
            "xres": xres,
        })
    return in_maps


_NC_CACHE = {}


def _get_nc(seq, dim, hh, n_cores, with_bhv=True):
    key = (seq, dim, hh, n_cores, with_bhv)
    if key not in _NC_CACHE:
        _NC_CACHE[key] = build_gau_nc(seq=seq, dim=dim, hh=hh,
                                      n_cores=n_cores, with_bhv=with_bhv)
    return _NC_CACHE[key]


def kernel(x, W_hidden, b_hidden, W_qk, b_qk, gamma_q, beta_q, gamma_k,
           beta_k, W_out, b_out):
    x = np.asarray(x)
    B, seq, dim = x.shape
    hh = W_hidden.shape[1] // 4
    n_cores = 2 * B
    with_bhv = bool(np.any(np.asarray(b_hidden)[: 2 * hh] != 0))
    nc = _get_nc(seq, dim, hh, n_cores, with_bhv=with_bhv)
    in_maps = make_in_maps(x, np.asarray(W_hidden), np.asarray(b_hidden),
                           np.asarray(W_qk), np.asarray(b_qk),
                           np.asarray(gamma_q), np.asarray(beta_q),
                           np.asarray(gamma_k), np.asarray(beta_k),
                           np.asarray(W_out), np.asarray(b_out),
                           n_cores=n_cores)
    res = run_bass_kernel_spmd(nc, in_maps, core_ids=list(range(n_cores)))
    out = np.empty((B, seq, dim), np.float32)
    for b in range(B):
        for h in range(2):
            out[b, own_rows(seq, h)] = res.results[2 * b + h]["out"]
    return out
